# revision 2
# baseline (speedup 1.0000x reference)
"""Trainium2 Bass kernel for nn_ContrastiveLoss (N=4096, D=128, NT=512, Q=8).

Strategy (8 NeuronCores, data parallel over N rows of x, no collective):
  - Each core owns R = N/8 = 512 rows of x (4 chunks of 128 partitions) and
    computes the two 512x4096 similarity blocks S_xx = x_chunk @ x.T and
    S_xy = x_chunk @ yf.T on the PE in fp8 (e4m3, K=D=128), streamed through
    one fused rhs tensor zT = [x.T | yf.T] of 8192 columns.
  - The only quantities the device must produce are the per-row totals
    totx[i] = sum_j exp(S_xx[ij]/T) (j != i) and toty[i] = sum_j exp(S_xy/T):
    everything data-dependent (own-track min/num, same-track and own-view
    exclusions) is a ~0.1%-of-FLOPs correction applied on the host in fp64.
  - exp + row-sum is split across two engines to beat the single-engine
    activation roofline (~34us on ACT alone):
      * ACT tiles: exp via the activation LUT with the fused per-partition
        accumulator (1 instruction per [128,2048] tile).
      * DVE tiles: Schraudolph bit-trick exp - int16 = round(A*s + B) IS the
        bf16 bit pattern of exp(s/T); one tensor_scalar (fp32 PSUM -> int16)
        computes it, a second 4x-mode tensor_scalar over the bf16-bitcast
        tile reduces it into the accumulator.
  - The diagonal self-terms exp(1/T) = e^20 would crush the fp32 accumulators
    (they are 25000x the rest of the row sum combined), so they are erased in
    PSUM by one extra 128-wide matmul per chunk: out += (-14*I).T @ I lands
    -14 exactly on the diagonal; exp((s-14)/T) underflows to +-0 in both the
    ACT and the Schraudolph path. x columns are rotated per-core so the
    diagonal block sits at a core-independent column (SPMD, single program).
  - Host combine: den = totx - same_track_x + toty - own_track_y, num from
    exact fp64 own-view dots; the N x N pair term mean log(den_j + num_i)
    collapses through a 4-term log1p series (num/den ~ 1e-6) with an exact
    numpy fallback.
"""

import numpy as np
import ml_dtypes

import concourse.bass as bass
import concourse.bacc as bacc
import concourse.tile as tile
import concourse.mybir as mybir
from concourse import bass_utils

P = 128           # partitions / rows per chunk
N = 4096          # total rows of x
D = 128           # feature dim
NT = 512          # number of tracks
Q = 8             # views per track
CORES = 8
R = N // CORES    # rows per core = 512
NCH = R // P      # chunks per core = 4
TEMP = 0.05
INV_T = 1.0 / TEMP
W = 2 * N         # fused rhs width: 4096 xx cols + 4096 xy cols
TW = 2048         # PSUM tile width (4 banks)
NTILES = W // TW  # 4 tiles per chunk: t=0,1 xx, t=2,3 xy
ZAP = 14.0        # diagonal kill: s_ii - 14 underflows exp safely in bf16 bits

# Schraudolph constants: int16 bits of bf16(exp(s/T)) ~= A*s + B
SCHRAUD_A = (128.0 / np.log(2.0)) * INV_T
SCHRAUD_B = 16248.65  # calibrated for zero mean relative error under RNE

F32 = mybir.dt.float32
BF16 = mybir.dt.bfloat16
I16 = mybir.dt.int16
FP8 = mybir.dt.float8e4
NP_FP8 = ml_dtypes.float8_e4m3
ALU = mybir.AluOpType
ACTF = mybir.ActivationFunctionType

_CACHE = {}


def _build():
    nc = bacc.Bacc("TRN2", target_bir_lowering=False, debug=False,
                   num_devices=CORES)

    zT_d = nc.dram_tensor("zT", [P, W], FP8, kind="ExternalInput")
    ipos_d = nc.dram_tensor("ipos", [P, P], FP8, kind="ExternalInput")
    ineg_d = nc.dram_tensor("ineg", [P, P], FP8, kind="ExternalInput")
    out_d = nc.dram_tensor("out", [P, NCH * NTILES], F32, kind="ExternalOutput")

    with tile.TileContext(nc) as tc:
        with (
            tc.tile_pool(name="persist", bufs=1) as pp,
            tc.tile_pool(name="ajunk", bufs=2) as ajp,
            tc.tile_pool(name="bits", bufs=2) as bp,
            tc.tile_pool(name="vjunk", bufs=2) as vjp,
            tc.tile_pool(name="psum", bufs=2, space="PSUM") as psp,
        ):
            zT_s = pp.tile([P, W], FP8, tag="zT_s")
            ipos_s = pp.tile([P, P], FP8, tag="ipos_s")
            ineg_s = pp.tile([P, P], FP8, tag="ineg_s")
            slots_s = pp.tile([P, NCH * NTILES], F32, tag="slots_s")
            warm_s = pp.tile([P, 8], F32, tag="warm_s")
            wout_s = pp.tile([P, 8], F32, tag="wout_s")

            # ---- input DMAs: identities first (tiny, needed by first zap),
            # then zT in consumption order split across both queue families.
            nc.gpsimd.dma_start(out=ipos_s[:], in_=ipos_d.ap())
            nc.gpsimd.dma_start(out=ineg_s[:], in_=ineg_d.ap())
            NSL = 8
            SW = W // NSL
            for k in range(NSL // 2):
                sl = slice(k * SW, (k + 1) * SW)
                nc.sync.dma_start(out=zT_s[:, sl], in_=zT_d.ap()[:, sl])
            for k in range(NSL // 2, NSL):
                sl = slice(k * SW, (k + 1) * SW)
                nc.gpsimd.dma_start(out=zT_s[:, sl], in_=zT_d.ap()[:, sl])

            # warm-up activation: pulls the exp table load to t=0
            nc.vector.memset(warm_s[:], 1.0)
            nc.scalar.activation(out=wout_s[:], in_=warm_s[:],
                                 func=ACTF.Exp, scale=1.0)

            for cc in range(NCH):
                lhsT = zT_s[:, cc * P:(cc + 1) * P]
                for t in range(NTILES):
                    ps = psp.tile([P, TW], F32, tag="ps")
                    base = TW * t
                    for k in range(TW // 512):
                        # keep the group open on the slice that gets the
                        # diagonal zap accumulated into it
                        zap_here = (t == 0 and k == 0)
                        nc.tensor.matmul(
                            out=ps[:, 512 * k:512 * (k + 1)],
                            lhsT=lhsT,
                            rhs=zT_s[:, base + 512 * k:base + 512 * (k + 1)],
                            start=True, stop=not zap_here,
                        )
                    if t == 0:
                        # erase the self-similarity diagonal: += (-14 I).T @ I
                        nc.tensor.matmul(
                            out=ps[:, cc * P:(cc + 1) * P],
                            lhsT=ineg_s[:],
                            rhs=ipos_s[:],
                            start=False, stop=True,
                        )
                    slot = slots_s[:, NTILES * cc + t:NTILES * cc + t + 1]
                    # engine routing: 7 tiles on DVE (Schraudolph), 9 on ACT
                    on_dve = (t in (0, 1)) and not (cc == 0 and t == 0)
                    if on_dve:
                        bits = bp.tile([P, TW], I16, tag="bits")
                        nc.vector.tensor_scalar(
                            out=bits[:], in0=ps[:],
                            scalar1=float(SCHRAUD_A), scalar2=float(SCHRAUD_B),
                            op0=ALU.mult, op1=ALU.add,
                        )
                        vj = vjp.tile([P, TW], BF16, tag="vjunk")
                        nc.vector.tensor_scalar(
                            out=vj[:], in0=bits[:].bitcast(BF16),
                            scalar1=1.0, scalar2=0.0,
                            op0=ALU.mult, op1=ALU.add,
                            accum_out=slot,
                        )
                    else:
                        aj = ajp.tile([P, TW], BF16, tag="ajunk")
                        nc.scalar.activation(
                            out=aj[:], in_=ps[:], func=ACTF.Exp,
                            scale=INV_T, accum_out=slot,
                        )

            nc.sync.dma_start(out=out_d.ap(), in_=slots_s[:])

    nc.compile()
    return nc


def get_nc():
    if "nc" not in _CACHE:
        _CACHE["nc"] = _build()
    return _CACHE["nc"]


def prepare_in_maps(x, track_idxs, y):
    x = np.ascontiguousarray(np.asarray(x), dtype=np.float32)
    y = np.ascontiguousarray(np.asarray(y), dtype=np.float32)
    xT8 = np.ascontiguousarray(x.T.astype(NP_FP8))            # [D, N]
    yT8 = np.ascontiguousarray(y.reshape(N, D).T.astype(NP_FP8))
    ipos = np.eye(P, dtype=NP_FP8)
    ineg = (-ZAP * np.eye(P)).astype(NP_FP8)
    in_maps = []
    for c in range(CORES):
        # rotate x columns so each core's own rows land at columns [0, 512)
        xroll = np.concatenate([xT8[:, c * R:], xT8[:, :c * R]], axis=1)
        zT = np.ascontiguousarray(np.concatenate([xroll, yT8], axis=1))
        in_maps.append({"zT": zT, "ipos": ipos, "ineg": ineg})
    return in_maps


def _host_corrections(x, track_idxs, y):
    """fp64 corrections: same-track xx exclusions, own-track y exclusions,
    sim_p. O(N*Q*D + sum_t c_t^2 * D) ~ 9 MFLOP."""
    x = np.asarray(x, dtype=np.float64)
    y = np.asarray(y, dtype=np.float64)
    t = np.asarray(track_idxs).astype(np.int64)

    # own-track views: dots[i, q] = x_i . y[t_i, q]
    yown = y[t]                                   # [N, Q, D]
    dots = np.einsum("nd,nqd->nq", x, yown)       # [N, Q]
    sim_p = dots.min(axis=1)                      # [N]
    own_sum = np.exp(dots * INV_T).sum(axis=1)    # [N]

    # same-track x pairs (excluding self): group rows by track
    samex = np.zeros(N, dtype=np.float64)
    order = np.argsort(t, kind="stable")
    ts = t[order]
    starts = np.searchsorted(ts, np.arange(NT), side="left")
    ends = np.searchsorted(ts, np.arange(NT), side="right")
    for tr in range(NT):
        idx = order[starts[tr]:ends[tr]]
        if idx.size < 2:
            continue
        xt = x[idx]
        G = np.exp((xt @ xt.T) * INV_T)
        np.fill_diagonal(G, 0.0)
        samex[idx] = G.sum(axis=1)
    return sim_p, own_sum, samex


def combine_outputs(outs, inputs):
    """outs: per-core [128, 16] accumulator tiles; row g = 512c + 128cc + p,
    slots [4cc + (0,1)] = xx halves, [4cc + (2,3)] = xy halves."""
    sim_p, own_sum, samex = _host_corrections(**inputs)

    totx = np.zeros(N, dtype=np.float64)
    toty = np.zeros(N, dtype=np.float64)
    for c, o in enumerate(outs):
        o = np.asarray(o, dtype=np.float64).reshape(P, NCH, NTILES)
        for cc in range(NCH):
            rows = slice(c * R + cc * P, c * R + (cc + 1) * P)
            totx[rows] = o[:, cc, 0] + o[:, cc, 1]
            toty[rows] = o[:, cc, 2] + o[:, cc, 3]

    den = (totx - samex) + (toty - own_sum)
    num = np.exp(sim_p * INV_T)
    if not (np.all(np.isfinite(den)) and np.all(den > 0)):
        return _exact_fallback(**inputs)

    # pair term mean_ij log(den_j + num_i) via log1p series in num/den
    logden = np.log(den)
    u_max = num.max() / den.min()
    if u_max < 1e-3:
        pair = N * logden.sum()
        term = 0.0
        for k in range(1, 5):
            term_k = ((-1.0) ** (k + 1) / k) * (num ** k).sum() * (den ** (-k)).sum()
            pair += term_k
            term = term_k
        if not abs(term) <= 1e-9 * abs(pair) + 1e-12:
            pair = np.log(den[None, :] + num[:, None]).sum()
    else:
        pair = np.log(den[None, :] + num[:, None]).sum()
    loss = pair / (N * N) - sim_p.mean() * INV_T
    return np.float32(loss)


def _exact_fallback(x, track_idxs, y):
    x = np.asarray(x, dtype=np.float64)
    y = np.asarray(y, dtype=np.float64)
    t = np.asarray(track_idxs)
    yf = y.reshape(NT * Q, D)
    ct = np.repeat(np.arange(NT), Q)
    own = t[:, None] == ct[None, :]
    S_xy = x @ yf.T
    sim_p = np.where(own, S_xy, np.inf).min(1)
    num = np.exp(sim_p / TEMP)
    den_y = np.where(own, 0.0, np.exp(S_xy / TEMP)).sum(1)
    same = t[:, None] == t[None, :]
    S_xx = x @ x.T
    den_x = np.where(same, 0.0, np.exp(S_xx / TEMP)).sum(1)
    den = den_y + den_x
    loss = np.log(den[None, :] + num[:, None]).mean() - (sim_p / TEMP).mean()
    return np.float32(loss)


def kernel(x, track_idxs, y):
    nc = get_nc()
    in_maps = prepare_in_maps(x, track_idxs, y)
    res = bass_utils.run_bass_kernel_spmd(nc, in_maps,
                                          core_ids=list(range(CORES)))
    return combine_outputs(
        [r["out"] for r in res.results],
        inputs={"x": x, "track_idxs": track_idxs, "y": y})


if __name__ == "__main__":
    nc = get_nc()
    print("build + compile OK")


# revision 5
# speedup vs baseline: 1.0017x; 1.0017x over previous
"""Trainium2 Bass kernel for nn_ContrastiveLoss (N=4096, D=128, NT=512, Q=8).

Strategy (8 NeuronCores, data parallel over N rows of x, no collective):
  - Each core owns R = N/8 = 512 rows of x (4 chunks of 128 partitions) and
    computes S_xx = x_chunk @ x.T and S_xy = x_chunk @ yf.T on the PE in fp8
    (e4m3, K=D=128) from one fused rhs tensor zT = [x.T | yf.T] (8192 cols).
  - The device only produces exp-row/column totals; everything data-dependent
    (own-track min/num, same-track and own-view exclusions) is a ~0.1%-of-
    FLOPs correction applied on the host in fp64.
  - exp work is split across engines to beat the single-engine activation
    roofline (~34us on ACT alone):
      * xy tiles -> ACT: exp LUT with the fused per-partition row accumulator.
      * xx tiles -> DVE: Schraudolph bit-trick exp: int16 = round(A*s + B)
        IS the bf16 bit pattern of exp(s/T); one tensor_scalar (fp32 PSUM ->
        int16) per tile. The row totals for xx come from PE ones-matmuls over
        the bf16-bitcast bits: S_xx is symmetric, so COLUMN sums of each
        [128 x 4096] slab are den_x contributions for the column-index rows.
        The 8 x 512-wide column-sum groups accumulate across all 4 chunks
        into two partition-packed PSUM banks (4 rows per bank at partitions
        0/32/64/96), then 8 tiny DMAs ship them out at the end.
  - The diagonal self-terms exp(1/T) = e^20 would crush the fp32 accumulators
    (25000x the rest of a row combined), so they are erased in PSUM by one
    extra 128-wide matmul per chunk: out += (-14*I).T @ I lands -14 exactly
    on the diagonal; exp((s-14)/T) underflows to +-0 in both exp paths.
    x columns are rotated per-core so the diagonal block sits at a
    core-independent column (pure SPMD, single program).
  - All bulk input DMA goes through the SWDGE (gpsimd) queue: it aggregates
    4 KB packets (~200 GB/s/core); the HWDGE path emits unaggregated 1 KB
    packets and trickles at ~30 GB/s on this runtime.
  - Host combine: den = totx - same_track_x + toty - own_track_y, num from
    exact fp64 own-view dots; the N x N pair-term mean log(den_j + num_i)
    collapses through a log1p series (num/den ~ 1e-6) with an exact fallback.
"""

import numpy as np
import ml_dtypes

import concourse.bass as bass
import concourse.bacc as bacc
import concourse.tile as tile
import concourse.mybir as mybir
from concourse import bass_utils

P = 128           # partitions / rows per chunk
N = 4096          # total rows of x
D = 128           # feature dim
NT = 512          # number of tracks
Q = 8             # views per track
CORES = 8
R = N // CORES    # rows per core = 512
NCH = R // P      # chunks per core = 4
TEMP = 0.05
INV_T = 1.0 / TEMP
W = 2 * N         # fused rhs width: 4096 xx cols + 4096 xy cols
TW = 1024         # PSUM tile width (2 banks)
NTILES = W // TW  # 8 tiles per chunk: t=0..3 xx, t=4..7 xy
NXX = N // TW     # 4 xx tiles per chunk
NSL = N // 512    # 8 column-sum slices per core
ZAP = 14.0        # diagonal kill: s_ii - 14 underflows exp safely in bf16 bits

# Schraudolph constants: int16 bits of bf16(exp(s/T)) ~= A*s + B (RNE convert)
SCHRAUD_A = (128.0 / np.log(2.0)) * INV_T
SCHRAUD_B = 16248.65  # calibrated for zero mean relative error

F32 = mybir.dt.float32
BF16 = mybir.dt.bfloat16
I16 = mybir.dt.int16
FP8 = mybir.dt.float8e4
NP_FP8 = ml_dtypes.float8_e4m3
ALU = mybir.AluOpType
ACTF = mybir.ActivationFunctionType

_CACHE = {}


def _build():
    nc = bacc.Bacc("TRN2", target_bir_lowering=False, debug=False,
                   num_devices=CORES)

    zT_d = nc.dram_tensor("zT", [P, W], FP8, kind="ExternalInput")
    ipos_d = nc.dram_tensor("ipos", [P, P], FP8, kind="ExternalInput")
    ineg_d = nc.dram_tensor("ineg", [P, P], FP8, kind="ExternalInput")
    # row sums of the xy tiles (per chunk x 4 tiles)
    slots_d = nc.dram_tensor("slots", [P, NCH * (NTILES - NXX)], F32,
                             kind="ExternalOutput")
    # column sums of the xx slabs: slice s -> row s
    sums_d = nc.dram_tensor("sums", [NSL, 512], F32, kind="ExternalOutput")

    with tile.TileContext(nc) as tc:
        with (
            tc.tile_pool(name="persist", bufs=1) as pp,
            tc.tile_pool(name="ajunk", bufs=2) as ajp,
            tc.tile_pool(name="bits", bufs=2) as bp,
            tc.tile_pool(name="psum", bufs=3, space="PSUM") as psp,
            tc.tile_pool(name="psums", bufs=1, space="PSUM") as psq,
        ):
            zT_s = pp.tile([P, W], FP8, tag="zT_s")
            ipos_s = pp.tile([P, P], FP8, tag="ipos_s")
            ineg_s = pp.tile([P, P], FP8, tag="ineg_s")
            ones_s = pp.tile([P, 1], BF16, tag="ones_s")
            slots_s = pp.tile([P, NCH * (NTILES - NXX)], F32, tag="slots_s")
            warm_s = pp.tile([P, 8], F32, tag="warm_s")
            wout_s = pp.tile([P, 8], F32, tag="wout_s")
            sumb0 = psq.tile([P, 512], F32, tag="sumb0")
            sumb1 = psq.tile([P, 512], F32, tag="sumb1")
            sumb = [sumb0, sumb1]

            # ---- input DMAs: identities via HWDGE (tiny); zT bulk via the
            # SWDGE queue in consumption order (4KB aggregated packets).
            nc.sync.dma_start(out=ipos_s[:], in_=ipos_d.ap())
            nc.sync.dma_start(out=ineg_s[:], in_=ineg_d.ap())
            for k in range(4):
                sl = slice(k * 2048, (k + 1) * 2048)
                nc.gpsimd.dma_start(out=zT_s[:, sl], in_=zT_d.ap()[:, sl])

            nc.vector.memset(ones_s[:], 1.0)
            # warm-up activation: pulls the exp table load to t=0
            nc.vector.memset(warm_s[:], 1.0)
            nc.scalar.activation(out=wout_s[:], in_=warm_s[:],
                                 func=ACTF.Exp, scale=1.0)

            for cc in range(NCH):
                lhsT = zT_s[:, cc * P:(cc + 1) * P]
                for t in range(NTILES):
                    ps = psp.tile([P, TW], F32, tag="ps")
                    base = TW * t
                    for k in range(TW // 512):
                        zap_here = (t == 0 and k == 0)
                        nc.tensor.matmul(
                            out=ps[:, 512 * k:512 * (k + 1)],
                            lhsT=lhsT,
                            rhs=zT_s[:, base + 512 * k:base + 512 * (k + 1)],
                            start=True, stop=not zap_here,
                        )
                    if t == 0:
                        # erase the self-similarity diagonal: += (-14 I).T @ I
                        nc.tensor.matmul(
                            out=ps[:, cc * P:(cc + 1) * P],
                            lhsT=ineg_s[:],
                            rhs=ipos_s[:],
                            start=False, stop=True,
                        )
                    if t < NXX:
                        # xx tile: Schraudolph exp on DVE, then PE column sums
                        bits = bp.tile([P, TW], I16, tag="bits")
                        nc.vector.tensor_scalar(
                            out=bits[:], in0=ps[:],
                            scalar1=float(SCHRAUD_A), scalar2=float(SCHRAUD_B),
                            op0=ALU.mult, op1=ALU.add,
                        )
                        for k in range(TW // 512):
                            s = t * (TW // 512) + k     # slice index 0..7
                            bq = s // 4
                            pq = 32 * (s % 4)
                            nc.tensor.matmul(
                                out=sumb[bq][pq:pq + 1, :],
                                lhsT=ones_s[:],
                                rhs=bits[:].bitcast(BF16)[:, 512 * k:512 * (k + 1)],
                                start=(cc == 0), stop=(cc == NCH - 1),
                                tile_position=(0, pq),
                            )
                    else:
                        # xy tile: ACT exp with fused row accumulator
                        slot_i = (NTILES - NXX) * cc + (t - NXX)
                        aj = ajp.tile([P, TW], BF16, tag="ajunk")
                        nc.scalar.activation(
                            out=aj[:], in_=ps[:], func=ACTF.Exp,
                            scale=INV_T,
                            accum_out=slots_s[:, slot_i:slot_i + 1],
                        )

            # ---- outputs: 8 column-sum rows + the xy slot tile
            sumc0 = pp.tile([P, 512], F32, tag="sumc0")
            sumc1 = pp.tile([P, 512], F32, tag="sumc1")
            nc.vector.tensor_copy(sumc0[:], sumb0[:])
            nc.vector.tensor_copy(sumc1[:], sumb1[:])
            for s in range(NSL):
                src = sumc0 if s < 4 else sumc1
                pq = 32 * (s % 4)
                nc.scalar.dma_start(out=sums_d.ap()[s:s + 1, :],
                                    in_=src[pq:pq + 1, :])
            nc.scalar.dma_start(out=slots_d.ap(), in_=slots_s[:])

    nc.compile()
    return nc


def get_nc():
    if "nc" not in _CACHE:
        _CACHE["nc"] = _build()
    return _CACHE["nc"]


def prepare_in_maps(x, track_idxs, y):
    x = np.ascontiguousarray(np.asarray(x), dtype=np.float32)
    y = np.ascontiguousarray(np.asarray(y), dtype=np.float32)
    xT8 = np.ascontiguousarray(x.T.astype(NP_FP8))            # [D, N]
    yT8 = np.ascontiguousarray(y.reshape(N, D).T.astype(NP_FP8))
    ipos = np.eye(P, dtype=NP_FP8)
    ineg = (-ZAP * np.eye(P)).astype(NP_FP8)
    in_maps = []
    for c in range(CORES):
        # rotate x columns so each core's own rows land at columns [0, 512)
        xroll = np.concatenate([xT8[:, c * R:], xT8[:, :c * R]], axis=1)
        zT = np.ascontiguousarray(np.concatenate([xroll, yT8], axis=1))
        in_maps.append({"zT": zT, "ipos": ipos, "ineg": ineg})
    return in_maps


def _host_corrections(x, track_idxs, y):
    """fp64 corrections: same-track xx exclusions, own-track y exclusions,
    sim_p. O(N*Q*D + sum_t c_t^2 * D) ~ 9 MFLOP."""
    x = np.asarray(x, dtype=np.float64)
    y = np.asarray(y, dtype=np.float64)
    t = np.asarray(track_idxs).astype(np.int64)

    yown = y[t]                                   # [N, Q, D]
    dots = np.einsum("nd,nqd->nq", x, yown)       # [N, Q]
    sim_p = dots.min(axis=1)                      # [N]
    own_sum = np.exp(dots * INV_T).sum(axis=1)    # [N]

    samex = np.zeros(N, dtype=np.float64)
    order = np.argsort(t, kind="stable")
    ts = t[order]
    starts = np.searchsorted(ts, np.arange(NT), side="left")
    ends = np.searchsorted(ts, np.arange(NT), side="right")
    for tr in range(NT):
        idx = order[starts[tr]:ends[tr]]
        if idx.size < 2:
            continue
        xt = x[idx]
        G = np.exp((xt @ xt.T) * INV_T)
        np.fill_diagonal(G, 0.0)
        samex[idx] = G.sum(axis=1)
    return sim_p, own_sum, samex


def combine_outputs(slot_outs, sum_outs, inputs):
    """slot_outs: per-core [128, 16] xy row-accumulators (row g = 512c +
    128cc + p, slot 4cc+j = xy cols [1024j, 1024j+1024)).
    sum_outs: per-core [8, 512] xx column sums; core c slice s col u is the
    den_x contribution of global row (512c + 512s + u) mod 4096."""
    sim_p, own_sum, samex = _host_corrections(**inputs)

    totx = np.zeros(N, dtype=np.float64)
    toty = np.zeros(N, dtype=np.float64)
    for c in range(CORES):
        o = np.asarray(slot_outs[c], dtype=np.float64).reshape(P, NCH, NTILES - NXX)
        for cc in range(NCH):
            rows = slice(c * R + cc * P, c * R + (cc + 1) * P)
            toty[rows] = o[:, cc, :].sum(axis=1)
        g = (c * R + np.arange(N)) % N
        totx[g] += np.asarray(sum_outs[c], dtype=np.float64).reshape(N)

    den = (totx - samex) + (toty - own_sum)
    num = np.exp(sim_p * INV_T)
    if not (np.all(np.isfinite(den)) and np.all(den > 0)):
        return _exact_fallback(**inputs)

    logden = np.log(den)
    if num.max() / den.min() < 1e-3:
        pair = N * logden.sum()
        term = 0.0
        for k in range(1, 5):
            term = ((-1.0) ** (k + 1) / k) * (num ** k).sum() * (den ** (-k)).sum()
            pair += term
        if not abs(term) <= 1e-9 * abs(pair) + 1e-12:
            pair = np.log(den[None, :] + num[:, None]).sum()
    else:
        pair = np.log(den[None, :] + num[:, None]).sum()
    loss = pair / (N * N) - sim_p.mean() * INV_T
    return np.float32(loss)


def _exact_fallback(x, track_idxs, y):
    x = np.asarray(x, dtype=np.float64)
    y = np.asarray(y, dtype=np.float64)
    t = np.asarray(track_idxs)
    yf = y.reshape(NT * Q, D)
    ct = np.repeat(np.arange(NT), Q)
    own = t[:, None] == ct[None, :]
    S_xy = x @ yf.T
    sim_p = np.where(own, S_xy, np.inf).min(1)
    num = np.exp(sim_p / TEMP)
    den_y = np.where(own, 0.0, np.exp(S_xy / TEMP)).sum(1)
    same = t[:, None] == t[None, :]
    S_xx = x @ x.T
    den_x = np.where(same, 0.0, np.exp(S_xx / TEMP)).sum(1)
    den = den_y + den_x
    loss = np.log(den[None, :] + num[:, None]).mean() - (sim_p / TEMP).mean()
    return np.float32(loss)


def kernel(x, track_idxs, y):
    nc = get_nc()
    in_maps = prepare_in_maps(x, track_idxs, y)
    res = bass_utils.run_bass_kernel_spmd(nc, in_maps,
                                          core_ids=list(range(CORES)))
    return combine_outputs(
        [r["slots"] for r in res.results],
        [r["sums"] for r in res.results],
        inputs={"x": x, "track_idxs": track_idxs, "y": y})


if __name__ == "__main__":
    nc = get_nc()
    print("build + compile OK")


# revision 8
# speedup vs baseline: 1.0563x; 1.0545x over previous
"""Trainium2 Bass kernel for nn_ContrastiveLoss (N=4096, D=128, NT=512, Q=8).

Strategy (8 NeuronCores, data parallel over N rows of x, no collective):
  - Each core owns R = N/8 = 512 rows of x (4 chunks of 128 partitions) and
    computes S_xx = x_chunk @ x.T and S_xy = x_chunk @ yf.T on the PE in fp8
    (e4m3, K=D=128) from one fused rhs tensor zT = [x.T | yf.T] (8192 cols).
  - The device only produces exp-row/column totals; everything data-dependent
    (own-track min/num, same-track and own-view exclusions) is a ~0.1%-of-
    FLOPs correction applied on the host in fp64.
  - exp work is split across engines to beat the single-engine activation
    roofline (~34us on ACT alone):
      * xy tiles -> ACT: exp LUT with the fused per-partition row accumulator.
      * xx tiles -> DVE: Schraudolph bit-trick exp: int16 = round(A*s + B)
        IS the bf16 bit pattern of exp(s/T); one tensor_scalar (fp32 PSUM ->
        int16) per tile. The row totals for xx come from PE ones-matmuls over
        the bf16-bitcast bits: S_xx is symmetric, so COLUMN sums of each
        [128 x 4096] slab are den_x contributions for the column-index rows.
        The 8 x 512-wide column-sum groups accumulate across all 4 chunks
        into two partition-packed PSUM banks (4 rows per bank at partitions
        0/32/64/96), then 8 tiny DMAs ship them out at the end.
  - The diagonal self-terms exp(1/T) = e^20 would crush the fp32 accumulators
    (25000x the rest of a row combined), so they are erased in PSUM by one
    extra 128-wide matmul per chunk: out += (-14*I).T @ I lands -14 exactly
    on the diagonal; exp((s-14)/T) underflows to +-0 in both exp paths.
    x columns are rotated per-core so the diagonal block sits at a
    core-independent column (pure SPMD, single program).
  - All bulk input DMA goes through the SWDGE (gpsimd) queue: it aggregates
    4 KB packets (~200 GB/s/core); the HWDGE path emits unaggregated 1 KB
    packets and trickles at ~30 GB/s on this runtime.
  - Host combine: den = totx - same_track_x + toty - own_track_y, num from
    exact fp64 own-view dots; the N x N pair-term mean log(den_j + num_i)
    collapses through a log1p series (num/den ~ 1e-6) with an exact fallback.
"""

import numpy as np
import ml_dtypes

import concourse.bass as bass
import concourse.bacc as bacc
import concourse.tile as tile
import concourse.mybir as mybir
from concourse import bass_utils

P = 128           # partitions / rows per chunk
N = 4096          # total rows of x
D = 128           # feature dim
NT = 512          # number of tracks
Q = 8             # views per track
CORES = 8
R = N // CORES    # rows per core = 512
NCH = R // P      # chunks per core = 4
TEMP = 0.05
INV_T = 1.0 / TEMP
W = 2 * N         # fused rhs width: 4096 xx cols + 4096 xy cols
TW = 1024         # PSUM tile width (2 banks)
NTILES = W // TW  # 8 tiles per chunk: t=0..3 xx, t=4..7 xy
NXX = N // TW     # 4 xx tiles per chunk
NSL = N // 512    # 8 column-sum slices per core
ZAP = 14.0        # diagonal kill: s_ii - 14 underflows exp safely in bf16 bits

# Schraudolph constants: int16 bits of bf16(exp(s/T)) ~= A*s + B (RNE convert)
SCHRAUD_A = (128.0 / np.log(2.0)) * INV_T
SCHRAUD_B = 16248.65  # calibrated for zero mean relative error

F32 = mybir.dt.float32
BF16 = mybir.dt.bfloat16
I16 = mybir.dt.int16
FP8 = mybir.dt.float8e4
NP_FP8 = ml_dtypes.float8_e4m3
ALU = mybir.AluOpType
ACTF = mybir.ActivationFunctionType

_CACHE = {}


def _build():
    nc = bacc.Bacc("TRN2", target_bir_lowering=False, debug=False,
                   num_devices=CORES)

    zT_d = nc.dram_tensor("zT", [P, W], FP8, kind="ExternalInput")
    ipos_d = nc.dram_tensor("ipos", [P, P], FP8, kind="ExternalInput")
    ineg_d = nc.dram_tensor("ineg", [P, P], FP8, kind="ExternalInput")
    # row sums of the xy tiles (per chunk x 4 tiles)
    slots_d = nc.dram_tensor("slots", [P, NCH * (NTILES - NXX)], F32,
                             kind="ExternalOutput")
    # column sums of the xx slabs: slice s -> row s
    sums_d = nc.dram_tensor("sums", [NSL, 512], F32, kind="ExternalOutput")

    with tile.TileContext(nc) as tc:
        with (
            tc.tile_pool(name="persist", bufs=1) as pp,
            tc.tile_pool(name="ajunk", bufs=2) as ajp,
            tc.tile_pool(name="bits", bufs=3) as bp,
            tc.tile_pool(name="psum", bufs=3, space="PSUM") as psp,
            tc.tile_pool(name="psums", bufs=1, space="PSUM") as psq,
        ):
            zT_s = pp.tile([P, W], FP8, tag="zT_s")
            ipos_s = pp.tile([P, P], FP8, tag="ipos_s")
            ineg_s = pp.tile([P, P], FP8, tag="ineg_s")
            ones_s = pp.tile([P, 1], BF16, tag="ones_s")
            slots_s = pp.tile([P, NCH * (NTILES - NXX)], F32, tag="slots_s")
            warm_s = pp.tile([P, 8], F32, tag="warm_s")
            wout_s = pp.tile([P, 8], F32, tag="wout_s")
            sumb0 = psq.tile([P, 512], F32, tag="sumb0")
            sumb1 = psq.tile([P, 512], F32, tag="sumb1")
            sumb = [sumb0, sumb1]

            # ---- input DMAs: identities via HWDGE (tiny); zT bulk via the
            # SWDGE queue in consumption order (4KB aggregated packets).
            nc.sync.dma_start(out=ipos_s[:], in_=ipos_d.ap())
            nc.sync.dma_start(out=ineg_s[:], in_=ineg_d.ap())
            # first slice small so the first matmul can start early
            bounds = [0, 512, 2048, 4096, 6144, W]
            for a, b in zip(bounds[:-1], bounds[1:]):
                nc.gpsimd.dma_start(out=zT_s[:, a:b], in_=zT_d.ap()[:, a:b])

            nc.vector.memset(ones_s[:], 1.0)
            # warm-up activation: pulls the exp table load to t=0
            nc.vector.memset(warm_s[:], 1.0)
            nc.scalar.activation(out=wout_s[:], in_=warm_s[:],
                                 func=ACTF.Exp, scale=1.0)

            for cc in range(NCH):
                lhsT = zT_s[:, cc * P:(cc + 1) * P]
                for t in range(NTILES):
                    ps = psp.tile([P, TW], F32, tag="ps")
                    base = TW * t
                    for k in range(TW // 512):
                        zap_here = (t == 0 and k == 0)
                        nc.tensor.matmul(
                            out=ps[:, 512 * k:512 * (k + 1)],
                            lhsT=lhsT,
                            rhs=zT_s[:, base + 512 * k:base + 512 * (k + 1)],
                            start=True, stop=not zap_here,
                        )
                    if t == 0:
                        # erase the self-similarity diagonal: += (-14 I).T @ I
                        nc.tensor.matmul(
                            out=ps[:, cc * P:(cc + 1) * P],
                            lhsT=ineg_s[:],
                            rhs=ipos_s[:],
                            start=False, stop=True,
                        )
                    if t < NXX:
                        # xx tile: Schraudolph exp on DVE, then PE column sums
                        bits = bp.tile([P, TW], I16, tag="bits")
                        nc.vector.tensor_scalar(
                            out=bits[:], in0=ps[:],
                            scalar1=float(SCHRAUD_A), scalar2=float(SCHRAUD_B),
                            op0=ALU.mult, op1=ALU.add,
                        )
                        for k in range(TW // 512):
                            s = t * (TW // 512) + k     # slice index 0..7
                            bq = s // 4
                            pq = 32 * (s % 4)
                            nc.tensor.matmul(
                                out=sumb[bq][pq:pq + 1, :],
                                lhsT=ones_s[:],
                                rhs=bits[:].bitcast(BF16)[:, 512 * k:512 * (k + 1)],
                                start=(cc == 0), stop=(cc == NCH - 1),
                                tile_position=(0, pq),
                            )
                    else:
                        # xy tile: ACT exp with fused row accumulator
                        slot_i = (NTILES - NXX) * cc + (t - NXX)
                        aj = ajp.tile([P, TW], BF16, tag="ajunk")
                        nc.scalar.activation(
                            out=aj[:], in_=ps[:], func=ACTF.Exp,
                            scale=INV_T,
                            accum_out=slots_s[:, slot_i:slot_i + 1],
                        )

            # ---- outputs: 8 column-sum rows + the xy slot tile, 3 DMAs on
            # the otherwise-idle sync queue (scalar-engine issue slots cost
            # ~600ns each of ACT time)
            sumc0 = pp.tile([P, 512], F32, tag="sumc0")
            sumc1 = pp.tile([P, 512], F32, tag="sumc1")
            nc.vector.tensor_copy(sumc0[:], sumb0[:])
            nc.vector.tensor_copy(sumc1[:], sumb1[:])
            nc.sync.dma_start(out=sums_d.ap()[0:4, :], in_=sumc0[0:97:32, :])
            nc.sync.dma_start(out=sums_d.ap()[4:8, :], in_=sumc1[0:97:32, :])
            nc.sync.dma_start(out=slots_d.ap(), in_=slots_s[:])

    nc.compile()
    return nc


def get_nc():
    if "nc" not in _CACHE:
        _CACHE["nc"] = _build()
    return _CACHE["nc"]


def prepare_in_maps(x, track_idxs, y):
    x = np.ascontiguousarray(np.asarray(x), dtype=np.float32)
    y = np.ascontiguousarray(np.asarray(y), dtype=np.float32)
    xT8 = np.ascontiguousarray(x.T.astype(NP_FP8))            # [D, N]
    yT8 = np.ascontiguousarray(y.reshape(N, D).T.astype(NP_FP8))
    ipos = np.eye(P, dtype=NP_FP8)
    ineg = (-ZAP * np.eye(P)).astype(NP_FP8)
    in_maps = []
    for c in range(CORES):
        # rotate x columns so each core's own rows land at columns [0, 512)
        xroll = np.concatenate([xT8[:, c * R:], xT8[:, :c * R]], axis=1)
        zT = np.ascontiguousarray(np.concatenate([xroll, yT8], axis=1))
        in_maps.append({"zT": zT, "ipos": ipos, "ineg": ineg})
    return in_maps


def _host_corrections(x, track_idxs, y):
    """fp64 corrections: same-track xx exclusions, own-track y exclusions,
    sim_p. O(N*Q*D + sum_t c_t^2 * D) ~ 9 MFLOP."""
    x = np.asarray(x, dtype=np.float64)
    y = np.asarray(y, dtype=np.float64)
    t = np.asarray(track_idxs).astype(np.int64)

    yown = y[t]                                   # [N, Q, D]
    dots = np.einsum("nd,nqd->nq", x, yown)       # [N, Q]
    sim_p = dots.min(axis=1)                      # [N]
    own_sum = np.exp(dots * INV_T).sum(axis=1)    # [N]

    samex = np.zeros(N, dtype=np.float64)
    order = np.argsort(t, kind="stable")
    ts = t[order]
    starts = np.searchsorted(ts, np.arange(NT), side="left")
    ends = np.searchsorted(ts, np.arange(NT), side="right")
    for tr in range(NT):
        idx = order[starts[tr]:ends[tr]]
        if idx.size < 2:
            continue
        xt = x[idx]
        G = np.exp((xt @ xt.T) * INV_T)
        np.fill_diagonal(G, 0.0)
        samex[idx] = G.sum(axis=1)
    return sim_p, own_sum, samex


def combine_outputs(slot_outs, sum_outs, inputs):
    """slot_outs: per-core [128, 16] xy row-accumulators (row g = 512c +
    128cc + p, slot 4cc+j = xy cols [1024j, 1024j+1024)).
    sum_outs: per-core [8, 512] xx column sums; core c slice s col u is the
    den_x contribution of global row (512c + 512s + u) mod 4096."""
    sim_p, own_sum, samex = _host_corrections(**inputs)

    totx = np.zeros(N, dtype=np.float64)
    toty = np.zeros(N, dtype=np.float64)
    for c in range(CORES):
        o = np.asarray(slot_outs[c], dtype=np.float64).reshape(P, NCH, NTILES - NXX)
        for cc in range(NCH):
            rows = slice(c * R + cc * P, c * R + (cc + 1) * P)
            toty[rows] = o[:, cc, :].sum(axis=1)
        g = (c * R + np.arange(N)) % N
        totx[g] += np.asarray(sum_outs[c], dtype=np.float64).reshape(N)

    den = (totx - samex) + (toty - own_sum)
    num = np.exp(sim_p * INV_T)
    if not (np.all(np.isfinite(den)) and np.all(den > 0)):
        return _exact_fallback(**inputs)

    logden = np.log(den)
    if num.max() / den.min() < 1e-3:
        pair = N * logden.sum()
        term = 0.0
        for k in range(1, 5):
            term = ((-1.0) ** (k + 1) / k) * (num ** k).sum() * (den ** (-k)).sum()
            pair += term
        if not abs(term) <= 1e-9 * abs(pair) + 1e-12:
            pair = np.log(den[None, :] + num[:, None]).sum()
    else:
        pair = np.log(den[None, :] + num[:, None]).sum()
    loss = pair / (N * N) - sim_p.mean() * INV_T
    return np.float32(loss)


def _exact_fallback(x, track_idxs, y):
    x = np.asarray(x, dtype=np.float64)
    y = np.asarray(y, dtype=np.float64)
    t = np.asarray(track_idxs)
    yf = y.reshape(NT * Q, D)
    ct = np.repeat(np.arange(NT), Q)
    own = t[:, None] == ct[None, :]
    S_xy = x @ yf.T
    sim_p = np.where(own, S_xy, np.inf).min(1)
    num = np.exp(sim_p / TEMP)
    den_y = np.where(own, 0.0, np.exp(S_xy / TEMP)).sum(1)
    same = t[:, None] == t[None, :]
    S_xx = x @ x.T
    den_x = np.where(same, 0.0, np.exp(S_xx / TEMP)).sum(1)
    den = den_y + den_x
    loss = np.log(den[None, :] + num[:, None]).mean() - (sim_p / TEMP).mean()
    return np.float32(loss)


def kernel(x, track_idxs, y):
    nc = get_nc()
    in_maps = prepare_in_maps(x, track_idxs, y)
    res = bass_utils.run_bass_kernel_spmd(nc, in_maps,
                                          core_ids=list(range(CORES)))
    return combine_outputs(
        [r["slots"] for r in res.results],
        [r["sums"] for r in res.results],
        inputs={"x": x, "track_idxs": track_idxs, "y": y})


if __name__ == "__main__":
    nc = get_nc()
    print("build + compile OK")


# revision 9
# speedup vs baseline: 1.1085x; 1.0494x over previous
"""Trainium2 Bass kernel for nn_ContrastiveLoss (N=4096, D=128, NT=512, Q=8).

Strategy (8 NeuronCores, data parallel over N rows of x, no collective):
  - Each core owns R = N/8 = 512 rows of x (4 chunks of 128 partitions) and
    computes S_xx = x_chunk @ x.T and S_xy = x_chunk @ yf.T on the PE in fp8
    (e4m3, K=D=128) from one fused rhs tensor zT = [x.T | yf.T] (8192 cols).
  - The device only produces exp-row/column totals; everything data-dependent
    (own-track min/num, same-track and own-view exclusions) is a ~0.1%-of-
    FLOPs correction applied on the host in fp64.
  - exp work is split across engines to beat the single-engine activation
    roofline (~34us on ACT alone):
      * xy tiles -> ACT: exp LUT with the fused per-partition row accumulator.
      * xx tiles -> DVE: Schraudolph bit-trick exp: int16 = round(A*s + B)
        IS the bf16 bit pattern of exp(s/T); one tensor_scalar (fp32 PSUM ->
        int16) per tile. The row totals for xx come from PE ones-matmuls over
        the bf16-bitcast bits: S_xx is symmetric, so COLUMN sums of each
        [128 x 4096] slab are den_x contributions for the column-index rows.
        The 8 x 512-wide column-sum groups accumulate across all 4 chunks
        into two partition-packed PSUM banks (4 rows per bank at partitions
        0/32/64/96), then 8 tiny DMAs ship them out at the end.
  - The diagonal self-terms exp(1/T) = e^20 would crush the fp32 accumulators
    (25000x the rest of a row combined), so they are erased in PSUM by one
    extra 128-wide matmul per chunk: out += (-14*I).T @ I lands -14 exactly
    on the diagonal; exp((s-14)/T) underflows to +-0 in both exp paths.
    x columns are rotated per-core so the diagonal block sits at a
    core-independent column (pure SPMD, single program).
  - All bulk input DMA goes through the SWDGE (gpsimd) queue: it aggregates
    4 KB packets (~200 GB/s/core); the HWDGE path emits unaggregated 1 KB
    packets and trickles at ~30 GB/s on this runtime.
  - Host combine: den = totx - same_track_x + toty - own_track_y, num from
    exact fp64 own-view dots; the N x N pair-term mean log(den_j + num_i)
    collapses through a log1p series (num/den ~ 1e-6) with an exact fallback.
"""

import numpy as np
import ml_dtypes

import concourse.bass as bass
import concourse.bacc as bacc
import concourse.tile as tile
import concourse.mybir as mybir
from concourse import bass_utils

P = 128           # partitions / rows per chunk
N = 4096          # total rows of x
D = 128           # feature dim
NT = 512          # number of tracks
Q = 8             # views per track
CORES = 8
R = N // CORES    # rows per core = 512
NCH = R // P      # chunks per core = 4
TEMP = 0.05
INV_T = 1.0 / TEMP
W = 2 * N         # fused rhs width: 4096 xx cols + 4096 xy cols
TW = 1024         # PSUM tile width (2 banks)
NTILES = W // TW  # 8 tiles per chunk: t=0..3 xx, t=4..7 xy
NXX = N // TW     # 4 xx tiles per chunk
NSL = N // 512    # 8 column-sum slices per core
ZAP = 14.0        # diagonal kill: s_ii - 14 underflows exp safely in bf16 bits

# Schraudolph constants: int16 bits of bf16(exp(s/T)) ~= A*s + B (RNE convert)
SCHRAUD_A = (128.0 / np.log(2.0)) * INV_T
SCHRAUD_B = 16248.65  # calibrated for zero mean relative error

F32 = mybir.dt.float32
BF16 = mybir.dt.bfloat16
I16 = mybir.dt.int16
FP8 = mybir.dt.float8e4
NP_FP8 = ml_dtypes.float8_e4m3
ALU = mybir.AluOpType
ACTF = mybir.ActivationFunctionType

_CACHE = {}


def _build():
    nc = bacc.Bacc("TRN2", target_bir_lowering=False, debug=False,
                   num_devices=CORES)

    zT_d = nc.dram_tensor("zT", [P, W], FP8, kind="ExternalInput")
    ipos_d = nc.dram_tensor("ipos", [P, P], FP8, kind="ExternalInput")
    ineg_d = nc.dram_tensor("ineg", [P, P], FP8, kind="ExternalInput")
    # row sums of the xy tiles (per chunk x 4 tiles)
    slots_d = nc.dram_tensor("slots", [P, NCH * (NTILES - NXX)], F32,
                             kind="ExternalOutput")
    # column sums of the xx slabs: slice s -> row s
    sums_d = nc.dram_tensor("sums", [NSL, 512], F32, kind="ExternalOutput")

    with tile.TileContext(nc) as tc:
        with (
            tc.tile_pool(name="persist", bufs=1) as pp,
            tc.tile_pool(name="ajunk", bufs=2) as ajp,
            tc.tile_pool(name="bits", bufs=3) as bp,
            tc.tile_pool(name="psum", bufs=3, space="PSUM") as psp,
            tc.tile_pool(name="psums", bufs=1, space="PSUM") as psq,
        ):
            zT_s = pp.tile([P, W], FP8, tag="zT_s")
            ipos_s = pp.tile([P, P], FP8, tag="ipos_s")
            ineg_s = pp.tile([P, P], FP8, tag="ineg_s")
            ones_s = pp.tile([P, 1], BF16, tag="ones_s")
            slots_s = pp.tile([P, NCH * (NTILES - NXX)], F32, tag="slots_s")
            warm_s = pp.tile([P, 8], F32, tag="warm_s")
            wout_s = pp.tile([P, 8], F32, tag="wout_s")
            sumb0 = psq.tile([P, 512], F32, tag="sumb0")
            sumb1 = psq.tile([P, 512], F32, tag="sumb1")
            sumb = [sumb0, sumb1]

            # ---- input DMAs: identities via HWDGE (tiny); zT bulk via the
            # SWDGE queue in consumption order (4KB aggregated packets).
            nc.sync.dma_start(out=ipos_s[:], in_=ipos_d.ap())
            nc.sync.dma_start(out=ineg_s[:], in_=ineg_d.ap())
            # first slice small so the first matmul can start early
            bounds = [0, 512, 2048, 4096, 6144, W]
            for a, b in zip(bounds[:-1], bounds[1:]):
                nc.gpsimd.dma_start(out=zT_s[:, a:b], in_=zT_d.ap()[:, a:b])

            nc.vector.memset(ones_s[:], 1.0)
            # warm-up activation: pulls the exp table load to t=0
            nc.vector.memset(warm_s[:], 1.0)
            nc.scalar.activation(out=wout_s[:], in_=warm_s[:],
                                 func=ACTF.Exp, scale=1.0)

            for cc in range(NCH):
                lhsT = zT_s[:, cc * P:(cc + 1) * P]
                # interleave xy (ACT) and xx (DVE) tiles so both exp engines
                # stream concurrently instead of alternating per phase
                for t in (4, 0, 5, 1, 6, 2, 7, 3):
                    ps = psp.tile([P, TW], F32, tag="ps")
                    base = TW * t
                    for k in range(TW // 512):
                        zap_here = (t == 0 and k == 0)
                        nc.tensor.matmul(
                            out=ps[:, 512 * k:512 * (k + 1)],
                            lhsT=lhsT,
                            rhs=zT_s[:, base + 512 * k:base + 512 * (k + 1)],
                            start=True, stop=not zap_here,
                        )
                    if t == 0:
                        # erase the self-similarity diagonal: += (-14 I).T @ I
                        nc.tensor.matmul(
                            out=ps[:, cc * P:(cc + 1) * P],
                            lhsT=ineg_s[:],
                            rhs=ipos_s[:],
                            start=False, stop=True,
                        )
                    if t < NXX:
                        # xx tile: Schraudolph exp on DVE, then PE column sums
                        bits = bp.tile([P, TW], I16, tag="bits")
                        nc.vector.tensor_scalar(
                            out=bits[:], in0=ps[:],
                            scalar1=float(SCHRAUD_A), scalar2=float(SCHRAUD_B),
                            op0=ALU.mult, op1=ALU.add,
                        )
                        for k in range(TW // 512):
                            s = t * (TW // 512) + k     # slice index 0..7
                            bq = s // 4
                            pq = 32 * (s % 4)
                            nc.tensor.matmul(
                                out=sumb[bq][pq:pq + 1, :],
                                lhsT=ones_s[:],
                                rhs=bits[:].bitcast(BF16)[:, 512 * k:512 * (k + 1)],
                                start=(cc == 0), stop=(cc == NCH - 1),
                                tile_position=(0, pq),
                            )
                    else:
                        # xy tile: ACT exp with fused row accumulator
                        slot_i = (NTILES - NXX) * cc + (t - NXX)
                        aj = ajp.tile([P, TW], BF16, tag="ajunk")
                        nc.scalar.activation(
                            out=aj[:], in_=ps[:], func=ACTF.Exp,
                            scale=INV_T,
                            accum_out=slots_s[:, slot_i:slot_i + 1],
                        )

            # ---- outputs: 8 column-sum rows + the xy slot tile, 3 DMAs on
            # the otherwise-idle sync queue (scalar-engine issue slots cost
            # ~600ns each of ACT time)
            sumc0 = pp.tile([P, 512], F32, tag="sumc0")
            sumc1 = pp.tile([P, 512], F32, tag="sumc1")
            nc.vector.tensor_copy(sumc0[:], sumb0[:])
            nc.vector.tensor_copy(sumc1[:], sumb1[:])
            nc.sync.dma_start(out=sums_d.ap()[0:4, :], in_=sumc0[0:97:32, :])
            nc.sync.dma_start(out=sums_d.ap()[4:8, :], in_=sumc1[0:97:32, :])
            nc.sync.dma_start(out=slots_d.ap(), in_=slots_s[:])

    nc.compile()
    return nc


def get_nc():
    if "nc" not in _CACHE:
        _CACHE["nc"] = _build()
    return _CACHE["nc"]


def prepare_in_maps(x, track_idxs, y):
    x = np.ascontiguousarray(np.asarray(x), dtype=np.float32)
    y = np.ascontiguousarray(np.asarray(y), dtype=np.float32)
    xT8 = np.ascontiguousarray(x.T.astype(NP_FP8))            # [D, N]
    yT8 = np.ascontiguousarray(y.reshape(N, D).T.astype(NP_FP8))
    ipos = np.eye(P, dtype=NP_FP8)
    ineg = (-ZAP * np.eye(P)).astype(NP_FP8)
    in_maps = []
    for c in range(CORES):
        # rotate x columns so each core's own rows land at columns [0, 512)
        xroll = np.concatenate([xT8[:, c * R:], xT8[:, :c * R]], axis=1)
        zT = np.ascontiguousarray(np.concatenate([xroll, yT8], axis=1))
        in_maps.append({"zT": zT, "ipos": ipos, "ineg": ineg})
    return in_maps


def _host_corrections(x, track_idxs, y):
    """fp64 corrections: same-track xx exclusions, own-track y exclusions,
    sim_p. O(N*Q*D + sum_t c_t^2 * D) ~ 9 MFLOP."""
    x = np.asarray(x, dtype=np.float64)
    y = np.asarray(y, dtype=np.float64)
    t = np.asarray(track_idxs).astype(np.int64)

    yown = y[t]                                   # [N, Q, D]
    dots = np.einsum("nd,nqd->nq", x, yown)       # [N, Q]
    sim_p = dots.min(axis=1)                      # [N]
    own_sum = np.exp(dots * INV_T).sum(axis=1)    # [N]

    samex = np.zeros(N, dtype=np.float64)
    order = np.argsort(t, kind="stable")
    ts = t[order]
    starts = np.searchsorted(ts, np.arange(NT), side="left")
    ends = np.searchsorted(ts, np.arange(NT), side="right")
    for tr in range(NT):
        idx = order[starts[tr]:ends[tr]]
        if idx.size < 2:
            continue
        xt = x[idx]
        G = np.exp((xt @ xt.T) * INV_T)
        np.fill_diagonal(G, 0.0)
        samex[idx] = G.sum(axis=1)
    return sim_p, own_sum, samex


def combine_outputs(slot_outs, sum_outs, inputs):
    """slot_outs: per-core [128, 16] xy row-accumulators (row g = 512c +
    128cc + p, slot 4cc+j = xy cols [1024j, 1024j+1024)).
    sum_outs: per-core [8, 512] xx column sums; core c slice s col u is the
    den_x contribution of global row (512c + 512s + u) mod 4096."""
    sim_p, own_sum, samex = _host_corrections(**inputs)

    totx = np.zeros(N, dtype=np.float64)
    toty = np.zeros(N, dtype=np.float64)
    for c in range(CORES):
        o = np.asarray(slot_outs[c], dtype=np.float64).reshape(P, NCH, NTILES - NXX)
        for cc in range(NCH):
            rows = slice(c * R + cc * P, c * R + (cc + 1) * P)
            toty[rows] = o[:, cc, :].sum(axis=1)
        g = (c * R + np.arange(N)) % N
        totx[g] += np.asarray(sum_outs[c], dtype=np.float64).reshape(N)

    den = (totx - samex) + (toty - own_sum)
    num = np.exp(sim_p * INV_T)
    if not (np.all(np.isfinite(den)) and np.all(den > 0)):
        return _exact_fallback(**inputs)

    logden = np.log(den)
    if num.max() / den.min() < 1e-3:
        pair = N * logden.sum()
        term = 0.0
        for k in range(1, 5):
            term = ((-1.0) ** (k + 1) / k) * (num ** k).sum() * (den ** (-k)).sum()
            pair += term
        if not abs(term) <= 1e-9 * abs(pair) + 1e-12:
            pair = np.log(den[None, :] + num[:, None]).sum()
    else:
        pair = np.log(den[None, :] + num[:, None]).sum()
    loss = pair / (N * N) - sim_p.mean() * INV_T
    return np.float32(loss)


def _exact_fallback(x, track_idxs, y):
    x = np.asarray(x, dtype=np.float64)
    y = np.asarray(y, dtype=np.float64)
    t = np.asarray(track_idxs)
    yf = y.reshape(NT * Q, D)
    ct = np.repeat(np.arange(NT), Q)
    own = t[:, None] == ct[None, :]
    S_xy = x @ yf.T
    sim_p = np.where(own, S_xy, np.inf).min(1)
    num = np.exp(sim_p / TEMP)
    den_y = np.where(own, 0.0, np.exp(S_xy / TEMP)).sum(1)
    same = t[:, None] == t[None, :]
    S_xx = x @ x.T
    den_x = np.where(same, 0.0, np.exp(S_xx / TEMP)).sum(1)
    den = den_y + den_x
    loss = np.log(den[None, :] + num[:, None]).mean() - (sim_p / TEMP).mean()
    return np.float32(loss)


def kernel(x, track_idxs, y):
    nc = get_nc()
    in_maps = prepare_in_maps(x, track_idxs, y)
    res = bass_utils.run_bass_kernel_spmd(nc, in_maps,
                                          core_ids=list(range(CORES)))
    return combine_outputs(
        [r["slots"] for r in res.results],
        [r["sums"] for r in res.results],
        inputs={"x": x, "track_idxs": track_idxs, "y": y})


if __name__ == "__main__":
    nc = get_nc()
    print("build + compile OK")


# revision 11
# speedup vs baseline: 1.2191x; 1.0998x over previous
"""Trainium2 Bass kernel for nn_ContrastiveLoss (N=4096, D=128, NT=512, Q=8).

Strategy (8 NeuronCores, data parallel over N rows of x, no collective):
  - Each core owns R = N/8 = 512 rows of x (4 chunks of 128 partitions) and
    computes S_xx = x_chunk @ x.T and S_xy = x_chunk @ yf.T on the PE in fp8
    (e4m3, K=D=128) from one fused rhs tensor zT = [x.T | yf.T] (8192 cols).
  - The device only produces exp-row/column totals; everything data-dependent
    (own-track min/num, same-track and own-view exclusions) is a ~0.1%-of-
    FLOPs correction applied on the host in fp64.
  - exp work is split across engines to beat the single-engine activation
    roofline (~34us on ACT alone):
      * xy tiles -> ACT: exp LUT with the fused per-partition row accumulator.
      * xx tiles -> DVE: Schraudolph bit-trick exp: int16 = round(A*s + B)
        IS the bf16 bit pattern of exp(s/T); one tensor_scalar (fp32 PSUM ->
        int16) per tile. The row totals for xx come from PE ones-matmuls over
        the bf16-bitcast bits: S_xx is symmetric, so COLUMN sums of each
        [128 x 4096] slab are den_x contributions for the column-index rows.
        The 8 x 512-wide column-sum groups accumulate across all 4 chunks
        into two partition-packed PSUM banks (4 rows per bank at partitions
        0/32/64/96), then 8 tiny DMAs ship them out at the end.
  - The diagonal self-terms exp(1/T) = e^20 would crush the fp32 accumulators
    (25000x the rest of a row combined), so they are erased in PSUM by one
    extra 128-wide matmul per chunk: out += (-14*I).T @ I lands -14 exactly
    on the diagonal; exp((s-14)/T) underflows to +-0 in both exp paths.
    x columns are rotated per-core so the diagonal block sits at a
    core-independent column (pure SPMD, single program).
  - All bulk input DMA goes through the SWDGE (gpsimd) queue: it aggregates
    4 KB packets (~200 GB/s/core); the HWDGE path emits unaggregated 1 KB
    packets and trickles at ~30 GB/s on this runtime.
  - Host combine: den = totx - same_track_x + toty - own_track_y, num from
    exact fp64 own-view dots; the N x N pair-term mean log(den_j + num_i)
    collapses through a log1p series (num/den ~ 1e-6) with an exact fallback.
"""

import numpy as np
import ml_dtypes

import concourse.bass as bass
import concourse.bacc as bacc
import concourse.tile as tile
import concourse.mybir as mybir
from concourse import bass_utils

P = 128           # partitions / rows per chunk
N = 4096          # total rows of x
D = 128           # feature dim
NT = 512          # number of tracks
Q = 8             # views per track
CORES = 8
R = N // CORES    # rows per core = 512
NCH = R // P      # chunks per core = 4
TEMP = 0.05
INV_T = 1.0 / TEMP
W = 2 * N         # fused rhs width: 4096 xx cols + 4096 xy cols
TW = 1024         # PSUM tile width (2 banks)
NTILES = W // TW  # 8 tiles per chunk: t=0..3 xx, t=4..7 xy
NXX = N // TW     # 4 xx tiles per chunk
NSL = N // 512    # 8 column-sum slices per core
ZAP = 14.0        # diagonal kill: s_ii - 14 underflows exp safely in bf16 bits

# Schraudolph constants: int16 bits of bf16(exp(s/T)) ~= A*s + B (RNE convert)
SCHRAUD_A = (128.0 / np.log(2.0)) * INV_T
SCHRAUD_B = 16248.65  # calibrated for zero mean relative error

F32 = mybir.dt.float32
BF16 = mybir.dt.bfloat16
I16 = mybir.dt.int16
FP8 = mybir.dt.float8e4
NP_FP8 = ml_dtypes.float8_e4m3
ALU = mybir.AluOpType
ACTF = mybir.ActivationFunctionType

_CACHE = {}


def _build():
    nc = bacc.Bacc("TRN2", target_bir_lowering=False, debug=False,
                   num_devices=CORES)

    zT_d = nc.dram_tensor("zT", [P, W], FP8, kind="ExternalInput")
    ipos_d = nc.dram_tensor("ipos", [P, P], FP8, kind="ExternalInput")
    ineg_d = nc.dram_tensor("ineg", [P, P], FP8, kind="ExternalInput")
    # row sums of the xy tiles (per chunk x 4 tiles)
    slots_d = nc.dram_tensor("slots", [P, NCH * (NTILES - NXX)], F32,
                             kind="ExternalOutput")
    # column sums of the xx slabs: slice s -> row s
    sums_d = nc.dram_tensor("sums", [NSL, 512], F32, kind="ExternalOutput")

    with tile.TileContext(nc) as tc:
        with (
            tc.tile_pool(name="persist", bufs=1) as pp,
            tc.tile_pool(name="ajunk", bufs=2) as ajp,
            tc.tile_pool(name="bits", bufs=3) as bp,
            tc.tile_pool(name="psum", bufs=3, space="PSUM") as psp,
            tc.tile_pool(name="psums", bufs=1, space="PSUM") as psq,
        ):
            zT_s = pp.tile([P, W], FP8, tag="zT_s")
            ipos_s = pp.tile([P, P], FP8, tag="ipos_s")
            ineg_s = pp.tile([P, P], FP8, tag="ineg_s")
            ones_s = pp.tile([P, 1], BF16, tag="ones_s")
            slots_s = pp.tile([P, NCH * (NTILES - NXX)], F32, tag="slots_s")
            warm_s = pp.tile([P, 8], F32, tag="warm_s")
            wout_s = pp.tile([P, 8], F32, tag="wout_s")
            sumb0 = psq.tile([P, 512], F32, tag="sumb0")
            sumb1 = psq.tile([P, 512], F32, tag="sumb1")
            sumb = [sumb0, sumb1]

            # ---- input DMAs: identities via HWDGE (tiny); zT bulk via the
            # SWDGE queue in consumption order (4KB aggregated packets).
            nc.sync.dma_start(out=ipos_s[:], in_=ipos_d.ap())
            nc.sync.dma_start(out=ineg_s[:], in_=ineg_d.ap())
            # slices in exact tile-consumption order (xx/xy interleaved),
            # with a small first slice so the first matmul starts early
            slices = [(0, 512), (512, 1024), (4096, 5120), (1024, 2048),
                      (5120, 6144), (2048, 3072), (6144, 7168),
                      (3072, 4096), (7168, 8192)]
            for a, b in slices:
                nc.gpsimd.dma_start(out=zT_s[:, a:b], in_=zT_d.ap()[:, a:b])

            nc.vector.memset(ones_s[:], 1.0)
            # warm-up activation: pulls the exp table load to t=0
            nc.vector.memset(warm_s[:], 1.0)
            nc.scalar.activation(out=wout_s[:], in_=warm_s[:],
                                 func=ACTF.Exp, scale=1.0)

            for cc in range(NCH):
                lhsT = zT_s[:, cc * P:(cc + 1) * P]
                # interleave xy (ACT) and xx (DVE) tiles so both exp engines
                # stream concurrently instead of alternating per phase
                for t in (0, 4, 1, 5, 2, 6, 3, 7):
                    ps = psp.tile([P, TW], F32, tag="ps")
                    base = TW * t
                    for k in range(TW // 512):
                        zap_here = (t == 0 and k == 0)
                        nc.tensor.matmul(
                            out=ps[:, 512 * k:512 * (k + 1)],
                            lhsT=lhsT,
                            rhs=zT_s[:, base + 512 * k:base + 512 * (k + 1)],
                            start=True, stop=not zap_here,
                        )
                    if t == 0:
                        # erase the self-similarity diagonal: += (-14 I).T @ I
                        nc.tensor.matmul(
                            out=ps[:, cc * P:(cc + 1) * P],
                            lhsT=ineg_s[:],
                            rhs=ipos_s[:],
                            start=False, stop=True,
                        )
                    if t < NXX:
                        # xx tile: Schraudolph exp on DVE, then PE column sums
                        bits = bp.tile([P, TW], I16, tag="bits")
                        nc.vector.tensor_scalar(
                            out=bits[:], in0=ps[:],
                            scalar1=float(SCHRAUD_A), scalar2=float(SCHRAUD_B),
                            op0=ALU.mult, op1=ALU.add,
                        )
                        for k in range(TW // 512):
                            s = t * (TW // 512) + k     # slice index 0..7
                            bq = s // 4
                            pq = 32 * (s % 4)
                            nc.tensor.matmul(
                                out=sumb[bq][pq:pq + 1, :],
                                lhsT=ones_s[:],
                                rhs=bits[:].bitcast(BF16)[:, 512 * k:512 * (k + 1)],
                                start=(cc == 0), stop=(cc == NCH - 1),
                                tile_position=(0, pq),
                            )
                    else:
                        # xy tile: ACT exp with fused row accumulator
                        slot_i = (NTILES - NXX) * cc + (t - NXX)
                        aj = ajp.tile([P, TW], BF16, tag="ajunk")
                        nc.scalar.activation(
                            out=aj[:], in_=ps[:], func=ACTF.Exp,
                            scale=INV_T,
                            accum_out=slots_s[:, slot_i:slot_i + 1],
                        )

            # ---- outputs: 8 column-sum rows + the xy slot tile, 3 DMAs on
            # the otherwise-idle sync queue (scalar-engine issue slots cost
            # ~600ns each of ACT time)
            sumc0 = pp.tile([P, 512], F32, tag="sumc0")
            sumc1 = pp.tile([P, 512], F32, tag="sumc1")
            nc.vector.tensor_copy(sumc0[:], sumb0[:])
            nc.vector.tensor_copy(sumc1[:], sumb1[:])
            nc.sync.dma_start(out=sums_d.ap()[0:4, :], in_=sumc0[0:97:32, :])
            nc.sync.dma_start(out=sums_d.ap()[4:8, :], in_=sumc1[0:97:32, :])
            nc.sync.dma_start(out=slots_d.ap(), in_=slots_s[:])

    nc.compile()
    return nc


def get_nc():
    if "nc" not in _CACHE:
        _CACHE["nc"] = _build()
    return _CACHE["nc"]


def prepare_in_maps(x, track_idxs, y):
    x = np.ascontiguousarray(np.asarray(x), dtype=np.float32)
    y = np.ascontiguousarray(np.asarray(y), dtype=np.float32)
    xT8 = np.ascontiguousarray(x.T.astype(NP_FP8))            # [D, N]
    yT8 = np.ascontiguousarray(y.reshape(N, D).T.astype(NP_FP8))
    ipos = np.eye(P, dtype=NP_FP8)
    ineg = (-ZAP * np.eye(P)).astype(NP_FP8)
    in_maps = []
    for c in range(CORES):
        # rotate x columns so each core's own rows land at columns [0, 512)
        xroll = np.concatenate([xT8[:, c * R:], xT8[:, :c * R]], axis=1)
        zT = np.ascontiguousarray(np.concatenate([xroll, yT8], axis=1))
        in_maps.append({"zT": zT, "ipos": ipos, "ineg": ineg})
    return in_maps


def _host_corrections(x, track_idxs, y):
    """fp64 corrections: same-track xx exclusions, own-track y exclusions,
    sim_p. O(N*Q*D + sum_t c_t^2 * D) ~ 9 MFLOP."""
    x = np.asarray(x, dtype=np.float64)
    y = np.asarray(y, dtype=np.float64)
    t = np.asarray(track_idxs).astype(np.int64)

    yown = y[t]                                   # [N, Q, D]
    dots = np.einsum("nd,nqd->nq", x, yown)       # [N, Q]
    sim_p = dots.min(axis=1)                      # [N]
    own_sum = np.exp(dots * INV_T).sum(axis=1)    # [N]

    samex = np.zeros(N, dtype=np.float64)
    order = np.argsort(t, kind="stable")
    ts = t[order]
    starts = np.searchsorted(ts, np.arange(NT), side="left")
    ends = np.searchsorted(ts, np.arange(NT), side="right")
    for tr in range(NT):
        idx = order[starts[tr]:ends[tr]]
        if idx.size < 2:
            continue
        xt = x[idx]
        G = np.exp((xt @ xt.T) * INV_T)
        np.fill_diagonal(G, 0.0)
        samex[idx] = G.sum(axis=1)
    return sim_p, own_sum, samex


def combine_outputs(slot_outs, sum_outs, inputs):
    """slot_outs: per-core [128, 16] xy row-accumulators (row g = 512c +
    128cc + p, slot 4cc+j = xy cols [1024j, 1024j+1024)).
    sum_outs: per-core [8, 512] xx column sums; core c slice s col u is the
    den_x contribution of global row (512c + 512s + u) mod 4096."""
    sim_p, own_sum, samex = _host_corrections(**inputs)

    totx = np.zeros(N, dtype=np.float64)
    toty = np.zeros(N, dtype=np.float64)
    for c in range(CORES):
        o = np.asarray(slot_outs[c], dtype=np.float64).reshape(P, NCH, NTILES - NXX)
        for cc in range(NCH):
            rows = slice(c * R + cc * P, c * R + (cc + 1) * P)
            toty[rows] = o[:, cc, :].sum(axis=1)
        g = (c * R + np.arange(N)) % N
        totx[g] += np.asarray(sum_outs[c], dtype=np.float64).reshape(N)

    den = (totx - samex) + (toty - own_sum)
    num = np.exp(sim_p * INV_T)
    if not (np.all(np.isfinite(den)) and np.all(den > 0)):
        return _exact_fallback(**inputs)

    logden = np.log(den)
    if num.max() / den.min() < 1e-3:
        pair = N * logden.sum()
        term = 0.0
        for k in range(1, 5):
            term = ((-1.0) ** (k + 1) / k) * (num ** k).sum() * (den ** (-k)).sum()
            pair += term
        if not abs(term) <= 1e-9 * abs(pair) + 1e-12:
            pair = np.log(den[None, :] + num[:, None]).sum()
    else:
        pair = np.log(den[None, :] + num[:, None]).sum()
    loss = pair / (N * N) - sim_p.mean() * INV_T
    return np.float32(loss)


def _exact_fallback(x, track_idxs, y):
    x = np.asarray(x, dtype=np.float64)
    y = np.asarray(y, dtype=np.float64)
    t = np.asarray(track_idxs)
    yf = y.reshape(NT * Q, D)
    ct = np.repeat(np.arange(NT), Q)
    own = t[:, None] == ct[None, :]
    S_xy = x @ yf.T
    sim_p = np.where(own, S_xy, np.inf).min(1)
    num = np.exp(sim_p / TEMP)
    den_y = np.where(own, 0.0, np.exp(S_xy / TEMP)).sum(1)
    same = t[:, None] == t[None, :]
    S_xx = x @ x.T
    den_x = np.where(same, 0.0, np.exp(S_xx / TEMP)).sum(1)
    den = den_y + den_x
    loss = np.log(den[None, :] + num[:, None]).mean() - (sim_p / TEMP).mean()
    return np.float32(loss)


def kernel(x, track_idxs, y):
    nc = get_nc()
    in_maps = prepare_in_maps(x, track_idxs, y)
    res = bass_utils.run_bass_kernel_spmd(nc, in_maps,
                                          core_ids=list(range(CORES)))
    return combine_outputs(
        [r["slots"] for r in res.results],
        [r["sums"] for r in res.results],
        inputs={"x": x, "track_idxs": track_idxs, "y": y})


if __name__ == "__main__":
    nc = get_nc()
    print("build + compile OK")
